# revision 18
# baseline (speedup 1.0000x reference)
"""Trainium2 Bass kernel for nn_AttentionMechanism (additive/Bahdanau attention).

reference:
    enc_p   = enc @ W_enc + b_enc                  # [B,S,H]
    dec_p   = dec @ W_dec + b_dec                  # [B,H]
    combined= tanh(enc_p + dec_p[:,None,:])        # [B,S,H]
    scores  = combined @ W_att[:,0] + b_att        # [B,S]  (b_att shift is a
                                                   #  softmax no-op -> dropped)
    scores  = where(mask, scores, -inf)
    weights = softmax(scores, axis=1)              # [B,S]
    context = einsum('bse,bs->be', enc, weights) @ W_ctx + b_ctx   # [B,D]
    returns (context, weights)

Sharding: data-parallel over batch, 32 batches per core on 8 cores.

Per-core plan (B_c=32 batches, groups of G=4 batches):
  phase 1: enc_pT tiles [h128, s512] = sum_ec  W_enc[ec,h].T @ encT[ec,s]
           (stationary = W_enc tiles, moving = transposed-enc tiles)
           fused tanh(psum + (dec_p+b_enc)[h]) on ScalarE (per-partition bias)
  phase 2: scores[1, s512] = sum_hc W_att[hc].T @ tanhC[hc, s]
           4 batches packed into PE column groups (tile_position)
  softmax: batched over the group's rows [4, 1024] (mask additive)
  phase 3: context[1, e512] = sum_sc wT[sc].T @ enc_nat[sc, e]
           (stationary = softmax-weight columns via on-chip PE transpose,
            moving = natural-layout enc tiles), 4 batches column-packed
  final:   context @ W_ctx + b_ctx as one [32,512] matmul (ctx transposed
           on-chip), one DMA per output.

Inputs are staged host-side in both layouts (encT for phase 1, enc natural
for phase 3), bf16, plus f32r (fp32 rounded to 11-bit mantissa) weights.
"""

import os

import numpy as np

B, S, E, D, H = 256, 1024, 512, 512, 512
NCORES = 8
BC = B // NCORES  # 32 batches per core
G = 4  # batches per compute group (PE column packing width)
NG = BC // G
NPAIR = 2  # DMA granularity: pairs of batches
EC = E // 128
HC = H // 128
DC = D // 128
SC = S // 128  # natural-layout s-chunks
SH = 2  # s halves of 512 for phases 1/2

NEG_INF = -1.0e30

_CACHE = {}


def _round_f32r(x: np.ndarray) -> np.ndarray:
    """Round fp32 to the PE's f32r format (11-bit mantissa, RNE)."""
    u = np.ascontiguousarray(x, dtype=np.float32).view(np.uint32)
    r = (u + np.uint32(0x7FF) + ((u >> np.uint32(12)) & np.uint32(1))) & np.uint32(
        0xFFFFF000
    )
    return r.view(np.float32)


def _build_nc(ng=NG, compile=True):
    import concourse.bacc as bacc
    import concourse.mybir as mybir
    import concourse.tile as tile
    from concourse.bass import ts
    from concourse.masks import make_identity

    F32 = mybir.dt.float32
    F32R = mybir.dt.float32r
    BF16 = mybir.dt.bfloat16
    ENC_DT = BF16  # enc_t tiles + W_enc (phase 1 matmul dtype)
    ENCN_DT = BF16  # enc_n tiles + wT (phase 3 matmul dtype)
    TANH_DT = F32R  # tanh tiles + W_att (phase 2 matmul dtype)
    AF = mybir.ActivationFunctionType

    nc = bacc.Bacc("TRN2", target_bir_lowering=False, debug=False)

    # --- DRAM I/O (per-core shard shapes) ---
    d_enc_t = nc.dram_tensor("enc_t", [BC, EC, 128, S], ENC_DT, kind="ExternalInput").ap()
    d_enc_n = nc.dram_tensor("enc_n", [BC, SC, 128, E], ENCN_DT, kind="ExternalInput").ap()
    d_w_enc = nc.dram_tensor("w_enc", [128, EC, H], ENC_DT, kind="ExternalInput").ap()
    d_w_att = nc.dram_tensor("w_att", [128, HC], TANH_DT, kind="ExternalInput").ap()
    d_dec_t = nc.dram_tensor("dec_t", [128, DC, BC], F32R, kind="ExternalInput").ap()
    d_w_dec = nc.dram_tensor("w_dec", [128, DC, H], F32R, kind="ExternalInput").ap()
    d_w_ctx = nc.dram_tensor("w_ctx", [128, EC, D], F32R, kind="ExternalInput").ap()
    d_bias_de = nc.dram_tensor("bias_de", [BC, H], F32, kind="ExternalInput").ap()
    d_bias_ctx = nc.dram_tensor("bias_ctx", [BC, D], F32, kind="ExternalInput").ap()
    # additive mask rows pre-spread to partitions {0,32,64,96} per group
    d_madd = nc.dram_tensor("madd", [NG, 128, S], F32, kind="ExternalInput").ap()

    d_ctx_out = nc.dram_tensor("ctx_out", [BC, D], F32, kind="ExternalOutput").ap()
    d_w_out = nc.dram_tensor("w_out", [BC, S], F32, kind="ExternalOutput").ap()

    with tile.TileContext(nc) as tc:
        with (
            tc.tile_pool(name="consts", bufs=1) as consts,
            tc.tile_pool(name="acc", bufs=1) as acc,
            tc.tile_pool(name="enc_t_pool", bufs=3) as enc_t_pool,
            tc.tile_pool(name="enc_n_pool", bufs=3) as enc_n_pool,
            tc.tile_pool(name="tanh_pool", bufs=4) as tanh_pool,
            tc.tile_pool(name="grp", bufs=2) as grp,
            tc.tile_pool(name="ps", bufs=2, space="PSUM") as ps,
        ):
            # --- constants ---
            w_enc_sb = consts.tile([128, EC, H], ENC_DT)
            nc.sync.dma_start(out=w_enc_sb, in_=d_w_enc)
            w_att_sb = consts.tile([128, HC], TANH_DT)
            nc.sync.dma_start(out=w_att_sb, in_=d_w_att)
            dec_t_sb = consts.tile([128, DC, BC], F32R)
            nc.sync.dma_start(out=dec_t_sb, in_=d_dec_t)
            w_dec_sb = consts.tile([128, DC, H], F32R)
            nc.sync.dma_start(out=w_dec_sb, in_=d_w_dec)
            w_ctx_sb = consts.tile([128, EC, D], F32R)
            nc.sync.dma_start(out=w_ctx_sb, in_=d_w_ctx)
            bias_de_sb = consts.tile([BC, H], F32)
            nc.sync.dma_start(out=bias_de_sb, in_=d_bias_de)
            bias_ctx_sb = consts.tile([BC, D], F32)
            nc.sync.dma_start(out=bias_ctx_sb, in_=d_bias_ctx)
            ident32 = consts.tile([BC, BC], F32)
            make_identity(nc, ident32)
            ident128 = consts.tile([128, 128], F32)
            make_identity(nc, ident128)
            ident1 = consts.tile([1, 1], F32)
            nc.vector.memset(ident1, 1.0)

            # --- long-lived tiles ---
            ct_sb = acc.tile([128, HC, BC], F32)  # (dec_p + b_enc + b_dec)^T
            ctxt_sb = acc.tile([128, EC, BC], F32R)  # context^T, built per group
            if ng < NG:
                # reduced (sim) builds leave later groups' columns unwritten
                nc.gpsimd.memset(ctxt_sb.bitcast(F32), 0.0)
            out_sb = acc.tile([BC, D], F32)
            ctxt_ps = ps.tile([128, EC, BC], F32, tag="ctxt", bufs=1)
            if ng < NG:
                nc.vector.memset(ctxt_ps, 0.0)

            # --- dec_p: C = dec @ W_dec + (b_dec + b_enc), then transpose ---
            c_ps = ps.tile([BC, H], F32, tag="p1")
            for dc in range(DC):
                nc.tensor.matmul(
                    c_ps,
                    dec_t_sb[:, dc, :],
                    w_dec_sb[:, dc, :],
                    start=(dc == 0),
                    stop=(dc == DC - 1),
                )
            c_sb = grp.tile([BC, H], F32, tag="c_sb")
            nc.vector.tensor_add(c_sb, c_ps, bias_de_sb)
            ct_ps = ps.tile([128, HC, BC], F32, tag="wt", bufs=1)
            for hc in range(HC):
                nc.tensor.transpose(ct_ps[:, hc, :], c_sb[:, ts(hc, 128)], ident32)
            nc.vector.tensor_copy(ct_sb, ct_ps)

            # --- main loop over groups of G batches ---
            # group-local row placement: batch j of the group lives on
            # partition 32*j (PE column-group packing constraint)
            for g in range(ng):
                enc_t_tiles = []
                enc_n_tiles = []
                for pi in range(G // NPAIR):
                    b0 = g * G + pi * NPAIR
                    enc_t_tile = enc_t_pool.tile(
                        [128, NPAIR, EC, S], ENC_DT, tag="enc_t"
                    )
                    nc.sync.dma_start(
                        out=enc_t_tile,
                        in_=d_enc_t[b0 : b0 + NPAIR].rearrange("b c p s -> p b c s"),
                    )
                    enc_t_tiles.append(enc_t_tile)
                    enc_n_tile = enc_n_pool.tile(
                        [128, NPAIR, SC, E], ENCN_DT, tag="enc_n"
                    )
                    nc.sync.dma_start(
                        out=enc_n_tile,
                        in_=d_enc_n[b0 : b0 + NPAIR].rearrange("b c p s -> p b c s"),
                    )
                    enc_n_tiles.append(enc_n_tile)

                # scores_g prefilled with the additive mask (zeros rows too)
                scores_g = grp.tile([128, S], F32, tag="scores")
                nc.sync.dma_start(out=scores_g, in_=d_madd[g])
                scr_flat = grp.tile([1, G * S], F32, tag="scr_flat", bufs=1)

                # phases 1+2 per s-half
                for sh in range(SH):
                    for j in range(G):
                        b = g * G + j
                        sc_ps = ps.tile([1, 512], F32, tag="sc", bufs=1)
                        tanh_t = tanh_pool.tile([128, HC, 512], TANH_DT, tag="tanh")
                        enc_t_tile = enc_t_tiles[j // NPAIR]
                        for hc in range(HC):
                            p1 = ps.tile([128, 512], F32, tag="p1")
                            for ec in range(EC):
                                nc.tensor.matmul(
                                    p1,
                                    w_enc_sb[:, ec, ts(hc, 128)],
                                    enc_t_tile[:, j % NPAIR, ec, ts(sh, 512)],
                                    start=(ec == 0),
                                    stop=(ec == EC - 1),
                                )
                            nc.scalar.activation(
                                out=tanh_t[:, hc, :],
                                in_=p1,
                                func=AF.Tanh,
                                bias=ct_sb[:, hc, b : b + 1],
                            )
                        for hc in range(HC):
                            nc.tensor.matmul(
                                sc_ps,
                                w_att_sb[:, hc : hc + 1],
                                tanh_t[:, hc, :],
                                start=(hc == 0),
                                stop=(hc == HC - 1),
                            )
                        nc.scalar.activation(
                            out=scr_flat[:, j * S + sh * 512 : j * S + sh * 512 + 512],
                            in_=sc_ps,
                            func=AF.Copy,
                        )

                # scatter score rows onto partitions {0,32,64,96} (+= mask)
                for j in range(G):
                    nc.gpsimd.dma_start(
                        out=scores_g[32 * j : 32 * j + 1, :],
                        in_=scr_flat[:, j * S : (j + 1) * S],
                        accum_op=mybir.AluOpType.add,
                    )

                # --- softmax (rows live on partitions {0,32,64,96}) ---
                negmax_g = grp.tile([128, 1], F32, tag="negmax")
                nc.vector.tensor_reduce(
                    negmax_g,
                    scores_g,
                    axis=mybir.AxisListType.X,
                    op=mybir.AluOpType.max,
                    negate=True,
                )
                esum_g = grp.tile([128, 1], F32, tag="esum")
                nc.scalar.activation(
                    out=scores_g,
                    in_=scores_g,
                    func=AF.Exp,
                    bias=negmax_g,
                    accum_out=esum_g,
                )
                rsum_g = grp.tile([128, 1], F32, tag="rsum")
                nc.vector.reciprocal(rsum_g, esum_g)
                wnorm_g = grp.tile([128, S], F32, tag="wnorm")
                nc.vector.tensor_scalar_mul(wnorm_g, in0=scores_g, scalar1=rsum_g)

                # weights output rows {0,32,64,96} -> DRAM
                nc.sync.dma_start(
                    out=d_w_out[g * G : (g + 1) * G, :],
                    in_=wnorm_g.rearrange("(a b) s -> a b s", b=32)[:, 0, :],
                )

                # --- transpose weights to [s, b] columns for phase 3 ---
                wt_sb = grp.tile([128, SC, 128], ENCN_DT, tag="wt_sb")
                for sc in range(SC):
                    wtp = ps.tile([128, 128], F32, tag="wt", bufs=1)
                    nc.tensor.transpose(wtp, wnorm_g[:, ts(sc, 128)], ident128)
                    nc.vector.tensor_copy(wt_sb[:, sc, :], wtp)

                # --- phase 3: context = sum_s w[s] * enc[s, :] ---
                ctx_flat = grp.tile([1, G * D], F32, tag="ctx_flat", bufs=1)
                for j in range(G):
                    ctx_ps = ps.tile([1, 512], F32, tag="ctx")
                    for sc in range(SC):
                        nc.tensor.matmul(
                            ctx_ps,
                            wt_sb[:, sc, 32 * j : 32 * j + 1],
                            enc_n_tiles[j // NPAIR][:, j % NPAIR, sc, :],
                            start=(sc == 0),
                            stop=(sc == SC - 1),
                        )
                    nc.scalar.activation(
                        out=ctx_flat[:, j * D : (j + 1) * D],
                        in_=ctx_ps,
                        func=AF.Copy,
                    )
                # context rows -> transposed columns of ctxt_ps
                for j in range(G):
                    b = g * G + j
                    for ec in range(EC):
                        nc.tensor.transpose(
                            ctxt_ps[:, ec, b : b + 1],
                            ctx_flat[:, j * D + ec * 128 : j * D + (ec + 1) * 128],
                            ident1,
                        )

            # --- final: out = ctx @ W_ctx + b_ctx ---
            nc.vector.tensor_copy(ctxt_sb, ctxt_ps)
            fin_ps = ps.tile([BC, D], F32, tag="fin", bufs=1)
            for ec in range(EC):
                nc.tensor.matmul(
                    fin_ps,
                    ctxt_sb[:, ec, :],
                    w_ctx_sb[:, ec, :],
                    start=(ec == 0),
                    stop=(ec == EC - 1),
                )
            nc.vector.tensor_add(out_sb, fin_ps, bias_ctx_sb)

            nc.sync.dma_start(out=d_ctx_out, in_=out_sb)

    if compile:
        nc.compile()
    return nc


def _get_nc():
    if "nc" not in _CACHE:
        _CACHE["nc"] = _build_nc()
    return _CACHE["nc"]


def _prepare_in_maps(
    encoder_outputs,
    decoder_state,
    attention_mask,
    W_enc,
    b_enc,
    W_dec,
    b_dec,
    W_att,
    b_att,
    W_ctx,
    b_ctx,
):
    import ml_dtypes

    bf16 = ml_dtypes.bfloat16

    enc = np.ascontiguousarray(np.asarray(encoder_outputs, dtype=np.float32))
    dec = np.ascontiguousarray(np.asarray(decoder_state, dtype=np.float32))
    mask = np.asarray(attention_mask)
    W_enc = np.asarray(W_enc, dtype=np.float32)
    b_enc = np.asarray(b_enc, dtype=np.float32)
    W_dec = np.asarray(W_dec, dtype=np.float32)
    b_dec = np.asarray(b_dec, dtype=np.float32)
    W_att = np.asarray(W_att, dtype=np.float32)
    W_ctx = np.asarray(W_ctx, dtype=np.float32)
    b_ctx = np.asarray(b_ctx, dtype=np.float32)

    # [B,S,E] -> transposed layout [B, EC, 128, S] and natural [B, SC, 128, E]
    enc_t = np.ascontiguousarray(enc.transpose(0, 2, 1)).astype(bf16)
    enc_t = enc_t.reshape(NCORES, BC, EC, 128, S)
    enc_n = enc.astype(bf16).reshape(NCORES, BC, SC, 128, E)

    w_enc_h = np.ascontiguousarray(
        W_enc.reshape(EC, 128, H).transpose(1, 0, 2)
    ).astype(bf16)
    w_att_h = _round_f32r(np.ascontiguousarray(W_att[:, 0].reshape(HC, 128).T))
    w_dec_h = _round_f32r(
        np.ascontiguousarray(W_dec.reshape(DC, 128, H).transpose(1, 0, 2))
    )
    w_ctx_h = _round_f32r(
        np.ascontiguousarray(W_ctx.reshape(EC, 128, D).transpose(1, 0, 2))
    )
    bias_de_h = np.ascontiguousarray(
        np.broadcast_to((b_enc + b_dec)[None, :], (BC, H))
    ).astype(np.float32)
    bias_ctx_h = np.ascontiguousarray(np.broadcast_to(b_ctx[None, :], (BC, D))).astype(
        np.float32
    )
    madd_rows = np.where(mask, np.float32(0.0), np.float32(NEG_INF)).astype(np.float32)
    madd_rows = madd_rows.reshape(NCORES, NG, G, S)
    madd = np.zeros((NCORES, NG, 128, S), np.float32)
    madd[:, :, ::32, :] = madd_rows

    # dec_t[core]: [128, DC, BC]
    dec_sh = dec.reshape(NCORES, BC, D)
    dec_t = np.ascontiguousarray(
        dec_sh.transpose(0, 2, 1).reshape(NCORES, DC, 128, BC).transpose(0, 2, 1, 3)
    )
    dec_t = _round_f32r(dec_t)

    in_maps = []
    for c in range(NCORES):
        in_maps.append(
            {
                "enc_t": enc_t[c],
                "enc_n": enc_n[c],
                "w_enc": w_enc_h,
                "w_att": w_att_h,
                "dec_t": dec_t[c],
                "w_dec": w_dec_h,
                "w_ctx": w_ctx_h,
                "bias_de": bias_de_h,
                "bias_ctx": bias_ctx_h,
                "madd": madd[c],
            }
        )
    return in_maps


def kernel(**inputs):
    from concourse.bass_utils import run_bass_kernel_spmd

    in_maps = _prepare_in_maps(**inputs)
    nc = _get_nc()
    trace = bool(int(os.environ.get("KERNEL_TRACE", "0")))
    kwargs = {}
    if trace:
        kwargs = {"trace": True, "tmpdir": os.environ.get("KERNEL_TRACE_DIR")}
    res = run_bass_kernel_spmd(nc, in_maps, core_ids=list(range(NCORES)), **kwargs)
    _CACHE["last_result"] = res

    context = np.concatenate([res.results[c]["ctx_out"] for c in range(NCORES)], axis=0)
    weights = np.concatenate([res.results[c]["w_out"] for c in range(NCORES)], axis=0)
    return context.astype(np.float32), weights.astype(np.float32)


# revision 19
# speedup vs baseline: 1.1327x; 1.1327x over previous
"""Trainium2 Bass kernel for nn_AttentionMechanism (additive/Bahdanau attention).

reference:
    enc_p   = enc @ W_enc + b_enc                  # [B,S,H]
    dec_p   = dec @ W_dec + b_dec                  # [B,H]
    combined= tanh(enc_p + dec_p[:,None,:])        # [B,S,H]
    scores  = combined @ W_att[:,0] + b_att        # [B,S]  (b_att shift is a
                                                   #  softmax no-op -> dropped)
    scores  = where(mask, scores, -inf)
    weights = softmax(scores, axis=1)              # [B,S]
    context = einsum('bse,bs->be', enc, weights) @ W_ctx + b_ctx   # [B,D]
    returns (context, weights)

Sharding: data-parallel over batch, 32 batches per core on 8 cores.

Per-core plan (B_c=32 batches, groups of G=4 batches):
  phase 1: enc_pT tiles [h128, s512] = sum_ec  W_enc[ec,h].T @ encT[ec,s]
           (stationary = W_enc tiles, moving = transposed-enc tiles)
           fused tanh(psum + (dec_p+b_enc)[h]) on ScalarE (per-partition bias)
  phase 2: scores[1, s512] = sum_hc W_att[hc].T @ tanhC[hc, s]
           4 batches packed into PE column groups (tile_position)
  softmax: batched over the group's rows [4, 1024] (mask additive)
  phase 3: context[1, e512] = sum_sc wT[sc].T @ enc_nat[sc, e]
           (stationary = softmax-weight columns via on-chip PE transpose,
            moving = natural-layout enc tiles), 4 batches column-packed
  final:   context @ W_ctx + b_ctx as one [32,512] matmul (ctx transposed
           on-chip), one DMA per output.

Inputs are staged host-side in both layouts (encT for phase 1, enc natural
for phase 3), bf16, plus f32r (fp32 rounded to 11-bit mantissa) weights.
"""

import os

import numpy as np

B, S, E, D, H = 256, 1024, 512, 512, 512
NCORES = 8
BC = B // NCORES  # 32 batches per core
G = 4  # batches per compute group (PE column packing width)
NG = BC // G
NPAIR = 2  # DMA granularity: pairs of batches
EC = E // 128
HC = H // 128
DC = D // 128
SC = S // 128  # natural-layout s-chunks
SH = 2  # s halves of 512 for phases 1/2

NEG_INF = -1.0e30

_CACHE = {}


def _round_f32r(x: np.ndarray) -> np.ndarray:
    """Round fp32 to the PE's f32r format (11-bit mantissa, RNE)."""
    u = np.ascontiguousarray(x, dtype=np.float32).view(np.uint32)
    r = (u + np.uint32(0x7FF) + ((u >> np.uint32(12)) & np.uint32(1))) & np.uint32(
        0xFFFFF000
    )
    return r.view(np.float32)


def _build_nc(ng=NG, compile=True):
    import concourse.bacc as bacc
    import concourse.mybir as mybir
    import concourse.tile as tile
    from concourse.bass import ts
    from concourse.masks import make_identity

    F32 = mybir.dt.float32
    F32R = mybir.dt.float32r
    BF16 = mybir.dt.bfloat16
    ENC_DT = BF16  # enc_t tiles + W_enc (phase 1 matmul dtype)
    ENCN_DT = BF16  # enc_n tiles + wT (phase 3 matmul dtype)
    TANH_DT = BF16  # tanh tiles + W_att (phase 2 matmul dtype)
    AF = mybir.ActivationFunctionType

    nc = bacc.Bacc("TRN2", target_bir_lowering=False, debug=False)

    # --- DRAM I/O (per-core shard shapes) ---
    d_enc_t = nc.dram_tensor("enc_t", [BC, EC, 128, S], ENC_DT, kind="ExternalInput").ap()
    d_enc_n = nc.dram_tensor("enc_n", [BC, SC, 128, E], ENCN_DT, kind="ExternalInput").ap()
    d_w_enc = nc.dram_tensor("w_enc", [128, EC, H], ENC_DT, kind="ExternalInput").ap()
    d_w_att = nc.dram_tensor("w_att", [128, HC], TANH_DT, kind="ExternalInput").ap()
    d_dec_t = nc.dram_tensor("dec_t", [128, DC, BC], F32R, kind="ExternalInput").ap()
    d_w_dec = nc.dram_tensor("w_dec", [128, DC, H], F32R, kind="ExternalInput").ap()
    d_w_ctx = nc.dram_tensor("w_ctx", [128, EC, D], F32R, kind="ExternalInput").ap()
    d_bias_de = nc.dram_tensor("bias_de", [BC, H], F32, kind="ExternalInput").ap()
    d_bias_ctx = nc.dram_tensor("bias_ctx", [BC, D], F32, kind="ExternalInput").ap()
    # additive mask rows pre-spread to partitions {0,32,64,96} per group
    d_madd = nc.dram_tensor("madd", [NG, 128, S], F32, kind="ExternalInput").ap()

    d_ctx_out = nc.dram_tensor("ctx_out", [BC, D], F32, kind="ExternalOutput").ap()
    d_w_out = nc.dram_tensor("w_out", [BC, S], F32, kind="ExternalOutput").ap()

    with tile.TileContext(nc) as tc:
        with (
            tc.tile_pool(name="consts", bufs=1) as consts,
            tc.tile_pool(name="acc", bufs=1) as acc,
            tc.tile_pool(name="enc_t_pool", bufs=3) as enc_t_pool,
            tc.tile_pool(name="enc_n_pool", bufs=3) as enc_n_pool,
            tc.tile_pool(name="tanh_pool", bufs=4) as tanh_pool,
            tc.tile_pool(name="grp", bufs=2) as grp,
            tc.tile_pool(name="ps", bufs=2, space="PSUM") as ps,
        ):
            # --- constants ---
            w_enc_sb = consts.tile([128, EC, H], ENC_DT)
            nc.sync.dma_start(out=w_enc_sb, in_=d_w_enc)
            w_att_sb = consts.tile([128, HC], TANH_DT)
            nc.sync.dma_start(out=w_att_sb, in_=d_w_att)
            dec_t_sb = consts.tile([128, DC, BC], F32R)
            nc.sync.dma_start(out=dec_t_sb, in_=d_dec_t)
            w_dec_sb = consts.tile([128, DC, H], F32R)
            nc.sync.dma_start(out=w_dec_sb, in_=d_w_dec)
            w_ctx_sb = consts.tile([128, EC, D], F32R)
            nc.sync.dma_start(out=w_ctx_sb, in_=d_w_ctx)
            bias_de_sb = consts.tile([BC, H], F32)
            nc.sync.dma_start(out=bias_de_sb, in_=d_bias_de)
            bias_ctx_sb = consts.tile([BC, D], F32)
            nc.sync.dma_start(out=bias_ctx_sb, in_=d_bias_ctx)
            ident32 = consts.tile([BC, BC], F32)
            make_identity(nc, ident32)
            ident128 = consts.tile([128, 128], F32)
            make_identity(nc, ident128)

            # --- long-lived tiles ---
            ct_sb = acc.tile([128, HC, BC], F32)  # (dec_p + b_enc + b_dec)^T
            ctxt_sb = acc.tile([128, EC, BC], F32R)  # context^T, built per group
            if ng < NG:
                # reduced (sim) builds leave later groups' columns unwritten
                nc.gpsimd.memset(ctxt_sb.bitcast(F32), 0.0)
            out_sb = acc.tile([BC, D], F32)

            # --- dec_p: C = dec @ W_dec + (b_dec + b_enc), then transpose ---
            c_ps = ps.tile([BC, H], F32, tag="p1")
            for dc in range(DC):
                nc.tensor.matmul(
                    c_ps,
                    dec_t_sb[:, dc, :],
                    w_dec_sb[:, dc, :],
                    start=(dc == 0),
                    stop=(dc == DC - 1),
                )
            c_sb = grp.tile([BC, H], F32, tag="c_sb")
            nc.vector.tensor_add(c_sb, c_ps, bias_de_sb)
            ct_ps = ps.tile([128, HC, BC], F32, tag="wt", bufs=1)
            for hc in range(HC):
                nc.tensor.transpose(ct_ps[:, hc, :], c_sb[:, ts(hc, 128)], ident32)
            nc.vector.tensor_copy(ct_sb, ct_ps)

            # --- main loop over groups of G batches ---
            # group-local row placement: batch j of the group lives on
            # partition 32*j (PE column-group packing constraint)
            for g in range(ng):
                enc_t_tiles = []
                enc_n_tiles = []
                for pi in range(G // NPAIR):
                    b0 = g * G + pi * NPAIR
                    enc_t_tile = enc_t_pool.tile(
                        [128, NPAIR, EC, S], ENC_DT, tag="enc_t"
                    )
                    nc.sync.dma_start(
                        out=enc_t_tile,
                        in_=d_enc_t[b0 : b0 + NPAIR].rearrange("b c p s -> p b c s"),
                    )
                    enc_t_tiles.append(enc_t_tile)
                    enc_n_tile = enc_n_pool.tile(
                        [128, NPAIR, SC, E], ENCN_DT, tag="enc_n"
                    )
                    nc.sync.dma_start(
                        out=enc_n_tile,
                        in_=d_enc_n[b0 : b0 + NPAIR].rearrange("b c p s -> p b c s"),
                    )
                    enc_n_tiles.append(enc_n_tile)

                scores_g = grp.tile([128, S], F32, tag="scores")
                nc.gpsimd.memset(scores_g, 0.0)
                madd_g = grp.tile([128, S], F32, tag="madd")
                nc.sync.dma_start(out=madd_g, in_=d_madd[g])

                # phases 1+2 per s-half; 4 batches packed into PE col groups
                sc_ps = ps.tile([128, S], F32, tag="sc", bufs=1)
                for sh in range(SH):
                    tanh_tiles = []
                    for j in range(G):
                        b = g * G + j
                        tanh_t = tanh_pool.tile([128, HC, 512], TANH_DT, tag="tanh")
                        enc_t_tile = enc_t_tiles[j // NPAIR]
                        for hc in range(HC):
                            p1 = ps.tile([128, 512], F32, tag="p1")
                            for ec in range(EC):
                                nc.tensor.matmul(
                                    p1,
                                    w_enc_sb[:, ec, ts(hc, 128)],
                                    enc_t_tile[:, j % NPAIR, ec, ts(sh, 512)],
                                    start=(ec == 0),
                                    stop=(ec == EC - 1),
                                )
                            nc.scalar.activation(
                                out=tanh_t[:, hc, :],
                                in_=p1,
                                func=AF.Tanh,
                                bias=ct_sb[:, hc, b : b + 1],
                            )
                        tanh_tiles.append(tanh_t)
                    for hc in range(HC):
                        for j in range(G):
                            nc.tensor.matmul(
                                sc_ps[32 * j : 32 * j + 1, ts(sh, 512)],
                                w_att_sb[:, hc : hc + 1],
                                tanh_tiles[j][:, hc, :],
                                start=(hc == 0),
                                stop=(hc == HC - 1),
                                tile_position=(0, 32 * j),
                                skip_group_check=True,
                            )
                for j in range(G):
                    nc.scalar.activation(
                        out=scores_g[32 * j : 32 * j + 1, :],
                        in_=sc_ps[32 * j : 32 * j + 1, :],
                        func=AF.Copy,
                    )
                nc.vector.tensor_add(scores_g, scores_g, madd_g)

                # --- softmax (rows live on partitions {0,32,64,96}) ---
                negmax_g = grp.tile([128, 1], F32, tag="negmax")
                nc.vector.tensor_reduce(
                    negmax_g,
                    scores_g,
                    axis=mybir.AxisListType.X,
                    op=mybir.AluOpType.max,
                    negate=True,
                )
                esum_g = grp.tile([128, 1], F32, tag="esum")
                nc.scalar.activation(
                    out=scores_g,
                    in_=scores_g,
                    func=AF.Exp,
                    bias=negmax_g,
                    accum_out=esum_g,
                )
                rsum_g = grp.tile([128, 1], F32, tag="rsum")
                nc.vector.reciprocal(rsum_g, esum_g)
                wnorm_g = grp.tile([128, S], F32, tag="wnorm")
                nc.vector.tensor_scalar_mul(wnorm_g, in0=scores_g, scalar1=rsum_g)

                # weights output rows {0,32,64,96} -> DRAM
                nc.sync.dma_start(
                    out=d_w_out[g * G : (g + 1) * G, :],
                    in_=wnorm_g.rearrange("(a b) s -> a b s", b=32)[:, 0, :],
                )

                # --- transpose weights to [s, b] columns for phase 3 ---
                wt_sb = grp.tile([128, SC, 128], ENCN_DT, tag="wt_sb")
                for sc in range(SC):
                    wtp = ps.tile([128, 128], F32, tag="wt", bufs=1)
                    nc.tensor.transpose(wtp, wnorm_g[:, ts(sc, 128)], ident128)
                    nc.vector.tensor_copy(wt_sb[:, sc, :], wtp)

                # --- phase 3: context = sum_s w[s] * enc[s, :], col packed ---
                ctx_ps = ps.tile([128, 512], F32, tag="ctx")
                for sc in range(SC):
                    for j in range(G):
                        nc.tensor.matmul(
                            ctx_ps[32 * j : 32 * j + 1, :],
                            wt_sb[:, sc, 32 * j : 32 * j + 1],
                            enc_n_tiles[j // NPAIR][:, j % NPAIR, sc, :],
                            start=(sc == 0),
                            stop=(sc == SC - 1),
                            tile_position=(0, 32 * j),
                            skip_group_check=True,
                        )
                ctx_sp = grp.tile([128, D], F32, tag="ctx_sp")
                nc.gpsimd.memset(ctx_sp, 0.0)
                for j in range(G):
                    nc.scalar.activation(
                        out=ctx_sp[32 * j : 32 * j + 1, :],
                        in_=ctx_ps[32 * j : 32 * j + 1, :],
                        func=AF.Copy,
                    )
                # context rows -> transposed columns of ctxt_sb
                for ec in range(EC):
                    ctp = ps.tile([128, 128], F32, tag="wt", bufs=1)
                    nc.tensor.transpose(ctp, ctx_sp[:, ts(ec, 128)], ident128)
                    nc.vector.tensor_copy(
                        ctxt_sb[:, ec, g * G : (g + 1) * G],
                        ctp.rearrange("p (a b) -> p a b", b=32)[:, :, 0],
                    )

            # --- final: out = ctx @ W_ctx + b_ctx ---
            fin_ps = ps.tile([BC, D], F32, tag="fin", bufs=1)
            for ec in range(EC):
                nc.tensor.matmul(
                    fin_ps,
                    ctxt_sb[:, ec, :],
                    w_ctx_sb[:, ec, :],
                    start=(ec == 0),
                    stop=(ec == EC - 1),
                )
            nc.vector.tensor_add(out_sb, fin_ps, bias_ctx_sb)

            nc.sync.dma_start(out=d_ctx_out, in_=out_sb)

    if compile:
        nc.compile()
    return nc


def _get_nc():
    if "nc" not in _CACHE:
        _CACHE["nc"] = _build_nc()
    return _CACHE["nc"]


def _prepare_in_maps(
    encoder_outputs,
    decoder_state,
    attention_mask,
    W_enc,
    b_enc,
    W_dec,
    b_dec,
    W_att,
    b_att,
    W_ctx,
    b_ctx,
):
    import ml_dtypes

    bf16 = ml_dtypes.bfloat16

    enc = np.ascontiguousarray(np.asarray(encoder_outputs, dtype=np.float32))
    dec = np.ascontiguousarray(np.asarray(decoder_state, dtype=np.float32))
    mask = np.asarray(attention_mask)
    W_enc = np.asarray(W_enc, dtype=np.float32)
    b_enc = np.asarray(b_enc, dtype=np.float32)
    W_dec = np.asarray(W_dec, dtype=np.float32)
    b_dec = np.asarray(b_dec, dtype=np.float32)
    W_att = np.asarray(W_att, dtype=np.float32)
    W_ctx = np.asarray(W_ctx, dtype=np.float32)
    b_ctx = np.asarray(b_ctx, dtype=np.float32)

    # [B,S,E] -> transposed layout [B, EC, 128, S] and natural [B, SC, 128, E]
    enc_t = np.ascontiguousarray(enc.transpose(0, 2, 1)).astype(bf16)
    enc_t = enc_t.reshape(NCORES, BC, EC, 128, S)
    enc_n = enc.astype(bf16).reshape(NCORES, BC, SC, 128, E)

    w_enc_h = np.ascontiguousarray(
        W_enc.reshape(EC, 128, H).transpose(1, 0, 2)
    ).astype(bf16)
    w_att_h = np.ascontiguousarray(W_att[:, 0].reshape(HC, 128).T).astype(bf16)
    w_dec_h = _round_f32r(
        np.ascontiguousarray(W_dec.reshape(DC, 128, H).transpose(1, 0, 2))
    )
    w_ctx_h = _round_f32r(
        np.ascontiguousarray(W_ctx.reshape(EC, 128, D).transpose(1, 0, 2))
    )
    bias_de_h = np.ascontiguousarray(
        np.broadcast_to((b_enc + b_dec)[None, :], (BC, H))
    ).astype(np.float32)
    bias_ctx_h = np.ascontiguousarray(np.broadcast_to(b_ctx[None, :], (BC, D))).astype(
        np.float32
    )
    madd_rows = np.where(mask, np.float32(0.0), np.float32(NEG_INF)).astype(np.float32)
    madd_rows = madd_rows.reshape(NCORES, NG, G, S)
    madd = np.zeros((NCORES, NG, 128, S), np.float32)
    madd[:, :, ::32, :] = madd_rows

    # dec_t[core]: [128, DC, BC]
    dec_sh = dec.reshape(NCORES, BC, D)
    dec_t = np.ascontiguousarray(
        dec_sh.transpose(0, 2, 1).reshape(NCORES, DC, 128, BC).transpose(0, 2, 1, 3)
    )
    dec_t = _round_f32r(dec_t)

    in_maps = []
    for c in range(NCORES):
        in_maps.append(
            {
                "enc_t": enc_t[c],
                "enc_n": enc_n[c],
                "w_enc": w_enc_h,
                "w_att": w_att_h,
                "dec_t": dec_t[c],
                "w_dec": w_dec_h,
                "w_ctx": w_ctx_h,
                "bias_de": bias_de_h,
                "bias_ctx": bias_ctx_h,
                "madd": madd[c],
            }
        )
    return in_maps


def kernel(**inputs):
    from concourse.bass_utils import run_bass_kernel_spmd

    in_maps = _prepare_in_maps(**inputs)
    nc = _get_nc()
    trace = bool(int(os.environ.get("KERNEL_TRACE", "0")))
    kwargs = {}
    if trace:
        kwargs = {"trace": True, "tmpdir": os.environ.get("KERNEL_TRACE_DIR")}
    res = run_bass_kernel_spmd(nc, in_maps, core_ids=list(range(NCORES)), **kwargs)
    _CACHE["last_result"] = res

    context = np.concatenate([res.results[c]["ctx_out"] for c in range(NCORES)], axis=0)
    weights = np.concatenate([res.results[c]["w_out"] for c in range(NCORES)], axis=0)
    return context.astype(np.float32), weights.astype(np.float32)


# revision 21
# speedup vs baseline: 1285.1807x; 1134.6254x over previous
"""Trainium2 Bass kernel for nn_AttentionMechanism (additive/Bahdanau attention).

reference:
    enc_p   = enc @ W_enc + b_enc                  # [B,S,H]
    dec_p   = dec @ W_dec + b_dec                  # [B,H]
    combined= tanh(enc_p + dec_p[:,None,:])        # [B,S,H]
    scores  = combined @ W_att[:,0] + b_att        # [B,S]  (b_att shift is a
                                                   #  softmax no-op -> dropped)
    scores  = where(mask, scores, -inf)
    weights = softmax(scores, axis=1)              # [B,S]
    context = einsum('bse,bs->be', enc, weights) @ W_ctx + b_ctx   # [B,D]
    returns (context, weights)

Sharding: data-parallel over batch, 32 batches per core on 8 cores.

Per-core plan (B_c=32 batches, groups of G=4 batches):
  phase 1: enc_pT tiles [h128, s512] = sum_ec  W_enc[ec,h].T @ encT[ec,s]
           (stationary = W_enc tiles, moving = transposed-enc tiles)
           fused tanh(psum + (dec_p+b_enc)[h]) on ScalarE (per-partition bias)
  phase 2: scores[1, s512] = sum_hc W_att[hc].T @ tanhC[hc, s]
           4 batches packed into PE column groups (tile_position)
  softmax: batched over the group's rows [4, 1024] (mask additive)
  phase 3: context[1, e512] = sum_sc wT[sc].T @ enc_nat[sc, e]
           (stationary = softmax-weight columns via on-chip PE transpose,
            moving = natural-layout enc tiles), 4 batches column-packed
  final:   context @ W_ctx + b_ctx as one [32,512] matmul (ctx transposed
           on-chip), one DMA per output.

Inputs are staged host-side in both layouts (encT for phase 1, enc natural
for phase 3), bf16, plus f32r (fp32 rounded to 11-bit mantissa) weights.
"""

import os

import numpy as np

B, S, E, D, H = 256, 1024, 512, 512, 512
NCORES = 8
BC = B // NCORES  # 32 batches per core
G = 4  # batches per compute group (PE column packing width)
NG = BC // G
NPAIR = 2  # DMA granularity: pairs of batches
EC = E // 128
HC = H // 128
DC = D // 128
SC = S // 128  # natural-layout s-chunks
SH = 2  # s halves of 512 for phases 1/2

NEG_INF = -1.0e30

_CACHE = {}


def _round_f32r(x: np.ndarray) -> np.ndarray:
    """Round fp32 to the PE's f32r format (11-bit mantissa, RNE)."""
    u = np.ascontiguousarray(x, dtype=np.float32).view(np.uint32)
    r = (u + np.uint32(0x7FF) + ((u >> np.uint32(12)) & np.uint32(1))) & np.uint32(
        0xFFFFF000
    )
    return r.view(np.float32)


def _build_nc(ng=NG, compile=True):
    import concourse.bacc as bacc
    import concourse.mybir as mybir
    import concourse.tile as tile
    from concourse.bass import ts
    from concourse.masks import make_identity

    F32 = mybir.dt.float32
    F32R = mybir.dt.float32r
    BF16 = mybir.dt.bfloat16
    ENC_DT = BF16  # enc_t tiles + W_enc (phase 1 matmul dtype)
    ENCN_DT = BF16  # enc_n tiles + wT (phase 3 matmul dtype)
    TANH_DT = BF16  # tanh tiles + W_att (phase 2 matmul dtype)
    AF = mybir.ActivationFunctionType

    nc = bacc.Bacc("TRN2", target_bir_lowering=False, debug=False)

    # --- DRAM I/O (per-core shard shapes) ---
    d_enc_t = nc.dram_tensor("enc_t", [BC, EC, 128, S], ENC_DT, kind="ExternalInput").ap()
    d_enc_n = nc.dram_tensor("enc_n", [BC, SC, 128, E], ENCN_DT, kind="ExternalInput").ap()
    d_w_enc = nc.dram_tensor("w_enc", [128, EC, H], ENC_DT, kind="ExternalInput").ap()
    d_w_att = nc.dram_tensor("w_att", [128, HC], TANH_DT, kind="ExternalInput").ap()
    d_dec_t = nc.dram_tensor("dec_t", [128, DC, BC], F32R, kind="ExternalInput").ap()
    d_w_dec = nc.dram_tensor("w_dec", [128, DC, H], F32R, kind="ExternalInput").ap()
    d_w_ctx = nc.dram_tensor("w_ctx", [128, EC, D], F32R, kind="ExternalInput").ap()
    d_bias_de = nc.dram_tensor("bias_de", [BC, H], F32, kind="ExternalInput").ap()
    d_bias_ctx = nc.dram_tensor("bias_ctx", [BC, D], F32, kind="ExternalInput").ap()
    # additive mask rows pre-spread to partitions {0,32,64,96} per group
    d_madd = nc.dram_tensor("madd", [NG, 128, S], F32, kind="ExternalInput").ap()

    d_ctx_out = nc.dram_tensor("ctx_out", [BC, D], F32, kind="ExternalOutput").ap()
    d_w_out = nc.dram_tensor("w_out", [BC, S], F32, kind="ExternalOutput").ap()

    with tile.TileContext(nc) as tc:
        with (
            tc.tile_pool(name="consts", bufs=1) as consts,
            tc.tile_pool(name="acc", bufs=1) as acc,
            tc.tile_pool(name="enc_t_pool", bufs=3) as enc_t_pool,
            tc.tile_pool(name="enc_n_pool", bufs=3) as enc_n_pool,
            tc.tile_pool(name="tanh_pool", bufs=6) as tanh_pool,
            tc.tile_pool(name="grp", bufs=2) as grp,
            tc.tile_pool(name="ps", bufs=2, space="PSUM") as ps,
        ):
            # --- constants ---
            w_enc_sb = consts.tile([128, EC, H], ENC_DT)
            nc.sync.dma_start(out=w_enc_sb, in_=d_w_enc)
            w_att_sb = consts.tile([128, HC], TANH_DT)
            nc.sync.dma_start(out=w_att_sb, in_=d_w_att)
            dec_t_sb = consts.tile([128, DC, BC], F32R)
            nc.sync.dma_start(out=dec_t_sb, in_=d_dec_t)
            w_dec_sb = consts.tile([128, DC, H], F32R)
            nc.sync.dma_start(out=w_dec_sb, in_=d_w_dec)
            w_ctx_sb = consts.tile([128, EC, D], F32R)
            nc.sync.dma_start(out=w_ctx_sb, in_=d_w_ctx)
            bias_de_sb = consts.tile([BC, H], F32)
            nc.sync.dma_start(out=bias_de_sb, in_=d_bias_de)
            bias_ctx_sb = consts.tile([BC, D], F32)
            nc.sync.dma_start(out=bias_ctx_sb, in_=d_bias_ctx)
            ident32 = consts.tile([BC, BC], F32)
            make_identity(nc, ident32)
            ident128 = consts.tile([128, 128], F32)
            make_identity(nc, ident128)

            # --- long-lived tiles ---
            ct_sb = acc.tile([128, HC, BC], F32)  # (dec_p + b_enc + b_dec)^T
            ctxt_sb = acc.tile([128, EC, BC], F32R)  # context^T, built per group
            if ng < NG:
                # reduced (sim) builds leave later groups' columns unwritten
                nc.gpsimd.memset(ctxt_sb.bitcast(F32), 0.0)
            out_sb = acc.tile([BC, D], F32)

            # --- dec_p: C = dec @ W_dec + (b_dec + b_enc), then transpose ---
            c_ps = ps.tile([BC, H], F32, tag="p1")
            for dc in range(DC):
                nc.tensor.matmul(
                    c_ps,
                    dec_t_sb[:, dc, :],
                    w_dec_sb[:, dc, :],
                    start=(dc == 0),
                    stop=(dc == DC - 1),
                )
            c_sb = grp.tile([BC, H], F32, tag="c_sb")
            nc.vector.tensor_add(c_sb, c_ps, bias_de_sb)
            ct_ps = ps.tile([128, HC, BC], F32, tag="wt", bufs=1)
            for hc in range(HC):
                nc.tensor.transpose(ct_ps[:, hc, :], c_sb[:, ts(hc, 128)], ident32)
            nc.vector.tensor_copy(ct_sb, ct_ps)

            # --- main loop over groups of G batches ---
            # group-local row placement: batch j of the group lives on
            # partition 32*j (PE column-group packing constraint)
            for g in range(ng):
                enc_t_tiles = []
                enc_n_tiles = []
                for pi in range(G // NPAIR):
                    b0 = g * G + pi * NPAIR
                    enc_t_tile = enc_t_pool.tile(
                        [128, NPAIR, EC, S], ENC_DT, tag="enc_t"
                    )
                    nc.sync.dma_start(
                        out=enc_t_tile,
                        in_=d_enc_t[b0 : b0 + NPAIR].rearrange("b c p s -> p b c s"),
                    )
                    enc_t_tiles.append(enc_t_tile)
                    enc_n_tile = enc_n_pool.tile(
                        [128, NPAIR, SC, E], ENCN_DT, tag="enc_n"
                    )
                    nc.sync.dma_start(
                        out=enc_n_tile,
                        in_=d_enc_n[b0 : b0 + NPAIR].rearrange("b c p s -> p b c s"),
                    )
                    enc_n_tiles.append(enc_n_tile)

                scores_g = grp.tile([128, S], F32, tag="scores")
                nc.gpsimd.memset(scores_g, 0.0)
                madd_g = grp.tile([128, S], F32, tag="madd")
                nc.sync.dma_start(out=madd_g, in_=d_madd[g])

                # phases 1+2 per s-half; 4 batches packed into PE col groups
                for sh in range(SH):
                    sc_ps = ps.tile([128, 512], F32, tag="sc", bufs=2)
                    tanh_tiles = []
                    for j in range(G):
                        b = g * G + j
                        tanh_t = tanh_pool.tile([128, HC, 512], TANH_DT, tag="tanh")
                        enc_t_tile = enc_t_tiles[j // NPAIR]
                        for hc in range(HC):
                            p1 = ps.tile([128, 512], F32, tag="p1")
                            for ec in range(EC):
                                nc.tensor.matmul(
                                    p1,
                                    w_enc_sb[:, ec, ts(hc, 128)],
                                    enc_t_tile[:, j % NPAIR, ec, ts(sh, 512)],
                                    start=(ec == 0),
                                    stop=(ec == EC - 1),
                                )
                            nc.scalar.activation(
                                out=tanh_t[:, hc, :],
                                in_=p1,
                                func=AF.Tanh,
                                bias=ct_sb[:, hc, b : b + 1],
                            )
                        tanh_tiles.append(tanh_t)
                    for hc in range(HC):
                        for j in range(G):
                            nc.tensor.matmul(
                                sc_ps[32 * j : 32 * j + 1, :],
                                w_att_sb[:, hc : hc + 1],
                                tanh_tiles[j][:, hc, :],
                                start=(hc == 0),
                                stop=(hc == HC - 1),
                                tile_position=(0, 32 * j),
                                skip_group_check=True,
                            )
                    for j in range(G):
                        nc.vector.tensor_copy(
                            scores_g[32 * j : 32 * j + 1, ts(sh, 512)],
                            sc_ps[32 * j : 32 * j + 1, :],
                        )
                nc.vector.tensor_add(scores_g, scores_g, madd_g)

                # --- softmax (rows live on partitions {0,32,64,96}) ---
                negmax_g = grp.tile([128, 1], F32, tag="negmax")
                nc.vector.tensor_reduce(
                    negmax_g,
                    scores_g,
                    axis=mybir.AxisListType.X,
                    op=mybir.AluOpType.max,
                    negate=True,
                )
                esum_g = grp.tile([128, 1], F32, tag="esum")
                nc.scalar.activation(
                    out=scores_g,
                    in_=scores_g,
                    func=AF.Exp,
                    bias=negmax_g,
                    accum_out=esum_g,
                )
                rsum_g = grp.tile([128, 1], F32, tag="rsum")
                nc.vector.reciprocal(rsum_g, esum_g)
                wnorm_g = grp.tile([128, S], F32, tag="wnorm")
                nc.vector.tensor_scalar_mul(wnorm_g, in0=scores_g, scalar1=rsum_g)

                # weights output rows {0,32,64,96} -> DRAM
                nc.sync.dma_start(
                    out=d_w_out[g * G : (g + 1) * G, :],
                    in_=wnorm_g.rearrange("(a b) s -> a b s", b=32)[:, 0, :],
                )

                # --- transpose weights to [s, b] columns for phase 3 ---
                wt_sb = grp.tile([128, SC, 128], ENCN_DT, tag="wt_sb")
                for sc in range(SC):
                    wtp = ps.tile([128, 128], F32, tag="wt", bufs=1)
                    nc.tensor.transpose(wtp, wnorm_g[:, ts(sc, 128)], ident128)
                    nc.vector.tensor_copy(wt_sb[:, sc, :], wtp)

                # --- phase 3: context = sum_s w[s] * enc[s, :], col packed ---
                ctx_ps = ps.tile([128, 512], F32, tag="ctx")
                for sc in range(SC):
                    for j in range(G):
                        nc.tensor.matmul(
                            ctx_ps[32 * j : 32 * j + 1, :],
                            wt_sb[:, sc, 32 * j : 32 * j + 1],
                            enc_n_tiles[j // NPAIR][:, j % NPAIR, sc, :],
                            start=(sc == 0),
                            stop=(sc == SC - 1),
                            tile_position=(0, 32 * j),
                            skip_group_check=True,
                        )
                ctx_sp = grp.tile([128, D], F32, tag="ctx_sp")
                nc.gpsimd.memset(ctx_sp, 0.0)
                for j in range(G):
                    nc.vector.tensor_copy(
                        ctx_sp[32 * j : 32 * j + 1, :],
                        ctx_ps[32 * j : 32 * j + 1, :],
                    )
                # context rows -> transposed columns of ctxt_sb
                for ec in range(EC):
                    ctp = ps.tile([128, 128], F32, tag="wt", bufs=1)
                    nc.tensor.transpose(ctp, ctx_sp[:, ts(ec, 128)], ident128)
                    nc.vector.tensor_copy(
                        ctxt_sb[:, ec, g * G : (g + 1) * G],
                        ctp.rearrange("p (a b) -> p a b", b=32)[:, :, 0],
                    )

            # --- final: out = ctx @ W_ctx + b_ctx ---
            fin_ps = ps.tile([BC, D], F32, tag="fin", bufs=1)
            for ec in range(EC):
                nc.tensor.matmul(
                    fin_ps,
                    ctxt_sb[:, ec, :],
                    w_ctx_sb[:, ec, :],
                    start=(ec == 0),
                    stop=(ec == EC - 1),
                )
            nc.vector.tensor_add(out_sb, fin_ps, bias_ctx_sb)

            nc.sync.dma_start(out=d_ctx_out, in_=out_sb)

    if compile:
        nc.compile()
    return nc


def _get_nc():
    if "nc" not in _CACHE:
        _CACHE["nc"] = _build_nc()
    return _CACHE["nc"]


def _prepare_in_maps(
    encoder_outputs,
    decoder_state,
    attention_mask,
    W_enc,
    b_enc,
    W_dec,
    b_dec,
    W_att,
    b_att,
    W_ctx,
    b_ctx,
):
    import ml_dtypes

    bf16 = ml_dtypes.bfloat16

    enc = np.ascontiguousarray(np.asarray(encoder_outputs, dtype=np.float32))
    dec = np.ascontiguousarray(np.asarray(decoder_state, dtype=np.float32))
    mask = np.asarray(attention_mask)
    W_enc = np.asarray(W_enc, dtype=np.float32)
    b_enc = np.asarray(b_enc, dtype=np.float32)
    W_dec = np.asarray(W_dec, dtype=np.float32)
    b_dec = np.asarray(b_dec, dtype=np.float32)
    W_att = np.asarray(W_att, dtype=np.float32)
    W_ctx = np.asarray(W_ctx, dtype=np.float32)
    b_ctx = np.asarray(b_ctx, dtype=np.float32)

    # [B,S,E] -> transposed layout [B, EC, 128, S] and natural [B, SC, 128, E]
    enc_t = np.ascontiguousarray(enc.transpose(0, 2, 1)).astype(bf16)
    enc_t = enc_t.reshape(NCORES, BC, EC, 128, S)
    enc_n = enc.astype(bf16).reshape(NCORES, BC, SC, 128, E)

    w_enc_h = np.ascontiguousarray(
        W_enc.reshape(EC, 128, H).transpose(1, 0, 2)
    ).astype(bf16)
    w_att_h = np.ascontiguousarray(W_att[:, 0].reshape(HC, 128).T).astype(bf16)
    w_dec_h = _round_f32r(
        np.ascontiguousarray(W_dec.reshape(DC, 128, H).transpose(1, 0, 2))
    )
    w_ctx_h = _round_f32r(
        np.ascontiguousarray(W_ctx.reshape(EC, 128, D).transpose(1, 0, 2))
    )
    bias_de_h = np.ascontiguousarray(
        np.broadcast_to((b_enc + b_dec)[None, :], (BC, H))
    ).astype(np.float32)
    bias_ctx_h = np.ascontiguousarray(np.broadcast_to(b_ctx[None, :], (BC, D))).astype(
        np.float32
    )
    madd_rows = np.where(mask, np.float32(0.0), np.float32(NEG_INF)).astype(np.float32)
    madd_rows = madd_rows.reshape(NCORES, NG, G, S)
    madd = np.zeros((NCORES, NG, 128, S), np.float32)
    madd[:, :, ::32, :] = madd_rows

    # dec_t[core]: [128, DC, BC]
    dec_sh = dec.reshape(NCORES, BC, D)
    dec_t = np.ascontiguousarray(
        dec_sh.transpose(0, 2, 1).reshape(NCORES, DC, 128, BC).transpose(0, 2, 1, 3)
    )
    dec_t = _round_f32r(dec_t)

    in_maps = []
    for c in range(NCORES):
        in_maps.append(
            {
                "enc_t": enc_t[c],
                "enc_n": enc_n[c],
                "w_enc": w_enc_h,
                "w_att": w_att_h,
                "dec_t": dec_t[c],
                "w_dec": w_dec_h,
                "w_ctx": w_ctx_h,
                "bias_de": bias_de_h,
                "bias_ctx": bias_ctx_h,
                "madd": madd[c],
            }
        )
    return in_maps


def kernel(**inputs):
    from concourse.bass_utils import run_bass_kernel_spmd

    in_maps = _prepare_in_maps(**inputs)
    nc = _get_nc()
    trace = bool(int(os.environ.get("KERNEL_TRACE", "0")))
    if not trace:
        # the axon NTFF trace path needs a hook module this container lacks
        os.environ.setdefault("BASS_NEVER_TRACE", "1")
    kwargs = {}
    if trace:
        kwargs = {"trace": True, "tmpdir": os.environ.get("KERNEL_TRACE_DIR")}
    res = run_bass_kernel_spmd(nc, in_maps, core_ids=list(range(NCORES)), **kwargs)
    _CACHE["last_result"] = res

    context = np.concatenate([res.results[c]["ctx_out"] for c in range(NCORES)], axis=0)
    weights = np.concatenate([res.results[c]["w_out"] for c in range(NCORES)], axis=0)
    return context.astype(np.float32), weights.astype(np.float32)


# revision 22
# speedup vs baseline: 1294.0303x; 1.0069x over previous
"""Trainium2 Bass kernel for nn_AttentionMechanism (additive/Bahdanau attention).

reference:
    enc_p   = enc @ W_enc + b_enc                  # [B,S,H]
    dec_p   = dec @ W_dec + b_dec                  # [B,H]
    combined= tanh(enc_p + dec_p[:,None,:])        # [B,S,H]
    scores  = combined @ W_att[:,0] + b_att        # [B,S]  (b_att shift is a
                                                   #  softmax no-op -> dropped)
    scores  = where(mask, scores, -inf)
    weights = softmax(scores, axis=1)              # [B,S]
    context = einsum('bse,bs->be', enc, weights) @ W_ctx + b_ctx   # [B,D]
    returns (context, weights)

Sharding: data-parallel over batch, 32 batches per core on 8 cores.

Per-core plan (B_c=32 batches, groups of G=4 batches):
  phase 1: enc_pT tiles [h128, s512] = sum_ec  W_enc[ec,h].T @ encT[ec,s]
           (stationary = W_enc tiles, moving = transposed-enc tiles)
           fused tanh(psum + (dec_p+b_enc)[h]) on ScalarE (per-partition bias)
  phase 2: scores[1, s512] = sum_hc W_att[hc].T @ tanhC[hc, s]
           4 batches packed into PE column groups (tile_position)
  softmax: batched over the group's rows [4, 1024] (mask additive)
  phase 3: context[1, e512] = sum_sc wT[sc].T @ enc_nat[sc, e]
           (stationary = softmax-weight columns via on-chip PE transpose,
            moving = natural-layout enc tiles), 4 batches column-packed
  final:   context @ W_ctx + b_ctx as one [32,512] matmul (ctx transposed
           on-chip), one DMA per output.

Inputs are staged host-side in both layouts (encT for phase 1, enc natural
for phase 3), bf16, plus f32r (fp32 rounded to 11-bit mantissa) weights.
"""

import os

import numpy as np

B, S, E, D, H = 256, 1024, 512, 512, 512
NCORES = 8
BC = B // NCORES  # 32 batches per core
G = 4  # batches per compute group (PE column packing width)
NG = BC // G
NPAIR = 2  # DMA granularity: pairs of batches
EC = E // 128
HC = H // 128
DC = D // 128
SC = S // 128  # natural-layout s-chunks
SH = 2  # s halves of 512 for phases 1/2

NEG_INF = -1.0e30

_CACHE = {}


def _round_f32r(x: np.ndarray) -> np.ndarray:
    """Round fp32 to the PE's f32r format (11-bit mantissa, RNE)."""
    u = np.ascontiguousarray(x, dtype=np.float32).view(np.uint32)
    r = (u + np.uint32(0x7FF) + ((u >> np.uint32(12)) & np.uint32(1))) & np.uint32(
        0xFFFFF000
    )
    return r.view(np.float32)


def _build_nc(ng=NG, compile=True):
    import concourse.bacc as bacc
    import concourse.mybir as mybir
    import concourse.tile as tile
    from concourse.bass import ts
    from concourse.masks import make_identity

    F32 = mybir.dt.float32
    F32R = mybir.dt.float32r
    BF16 = mybir.dt.bfloat16
    ENC_DT = BF16  # enc_t tiles + W_enc (phase 1 matmul dtype)
    ENCN_DT = BF16  # enc_n tiles + wT (phase 3 matmul dtype)
    TANH_DT = BF16  # tanh tiles + W_att (phase 2 matmul dtype)
    AF = mybir.ActivationFunctionType

    nc = bacc.Bacc("TRN2", target_bir_lowering=False, debug=False)

    # --- DRAM I/O (per-core shard shapes) ---
    d_enc_t = nc.dram_tensor("enc_t", [BC, EC, 128, S], ENC_DT, kind="ExternalInput").ap()
    d_enc_n = nc.dram_tensor("enc_n", [BC, SC, 128, E], ENCN_DT, kind="ExternalInput").ap()
    d_w_enc = nc.dram_tensor("w_enc", [128, EC, H], ENC_DT, kind="ExternalInput").ap()
    d_w_att = nc.dram_tensor("w_att", [128, HC], TANH_DT, kind="ExternalInput").ap()
    d_dec_t = nc.dram_tensor("dec_t", [128, DC, BC], F32R, kind="ExternalInput").ap()
    d_w_dec = nc.dram_tensor("w_dec", [128, DC, H], F32R, kind="ExternalInput").ap()
    d_w_ctx = nc.dram_tensor("w_ctx", [128, EC, D], F32R, kind="ExternalInput").ap()
    d_bias_de = nc.dram_tensor("bias_de", [BC, H], F32, kind="ExternalInput").ap()
    d_bias_ctx = nc.dram_tensor("bias_ctx", [BC, D], F32, kind="ExternalInput").ap()
    # additive mask rows pre-spread to partitions {0,32,64,96} per group
    d_madd = nc.dram_tensor("madd", [NG, 128, S], F32, kind="ExternalInput").ap()

    d_ctx_out = nc.dram_tensor("ctx_out", [BC, D], F32, kind="ExternalOutput").ap()
    d_w_out = nc.dram_tensor("w_out", [BC, S], F32, kind="ExternalOutput").ap()

    with tile.TileContext(nc) as tc:
        with (
            tc.tile_pool(name="consts", bufs=1) as consts,
            tc.tile_pool(name="acc", bufs=1) as acc,
            tc.tile_pool(name="enc_t_pool", bufs=12) as enc_t_pool,
            tc.tile_pool(name="enc_n_pool", bufs=3) as enc_n_pool,
            tc.tile_pool(name="tanh_pool", bufs=6) as tanh_pool,
            tc.tile_pool(name="grp", bufs=2) as grp,
            tc.tile_pool(name="ps", bufs=2, space="PSUM") as ps,
        ):
            # --- constants ---
            w_enc_sb = consts.tile([128, EC, H], ENC_DT)
            nc.sync.dma_start(out=w_enc_sb, in_=d_w_enc)
            w_att_sb = consts.tile([128, HC], TANH_DT)
            nc.sync.dma_start(out=w_att_sb, in_=d_w_att)
            dec_t_sb = consts.tile([128, DC, BC], F32R)
            nc.sync.dma_start(out=dec_t_sb, in_=d_dec_t)
            w_dec_sb = consts.tile([128, DC, H], F32R)
            nc.sync.dma_start(out=w_dec_sb, in_=d_w_dec)
            w_ctx_sb = consts.tile([128, EC, D], F32R)
            nc.sync.dma_start(out=w_ctx_sb, in_=d_w_ctx)
            bias_de_sb = consts.tile([BC, H], F32)
            nc.sync.dma_start(out=bias_de_sb, in_=d_bias_de)
            bias_ctx_sb = consts.tile([BC, D], F32)
            nc.sync.dma_start(out=bias_ctx_sb, in_=d_bias_ctx)
            ident32 = consts.tile([BC, BC], F32)
            make_identity(nc, ident32)
            ident128 = consts.tile([128, 128], F32)
            make_identity(nc, ident128)

            # --- long-lived tiles ---
            ct_sb = acc.tile([128, HC, BC], F32)  # (dec_p + b_enc + b_dec)^T
            ctxt_sb = acc.tile([128, EC, BC], F32R)  # context^T, built per group
            if ng < NG:
                # reduced (sim) builds leave later groups' columns unwritten
                nc.gpsimd.memset(ctxt_sb.bitcast(F32), 0.0)
            out_sb = acc.tile([BC, D], F32)

            # --- dec_p: C = dec @ W_dec + (b_dec + b_enc), then transpose ---
            c_ps = ps.tile([BC, H], F32, tag="p1")
            for dc in range(DC):
                nc.tensor.matmul(
                    c_ps,
                    dec_t_sb[:, dc, :],
                    w_dec_sb[:, dc, :],
                    start=(dc == 0),
                    stop=(dc == DC - 1),
                )
            c_sb = grp.tile([BC, H], F32, tag="c_sb")
            nc.vector.tensor_add(c_sb, c_ps, bias_de_sb)
            ct_ps = ps.tile([128, HC, BC], F32, tag="wt", bufs=1)
            for hc in range(HC):
                nc.tensor.transpose(ct_ps[:, hc, :], c_sb[:, ts(hc, 128)], ident32)
            nc.vector.tensor_copy(ct_sb, ct_ps)

            # --- main loop over groups of G batches ---
            # group-local row placement: batch j of the group lives on
            # partition 32*j (PE column-group packing constraint)
            for g in range(ng):
                enc_t_tiles = []  # [pair][ec] -> [128, NPAIR, S] tile
                enc_n_tiles = []
                for pi in range(G // NPAIR):
                    b0 = g * G + pi * NPAIR
                    chunks = []
                    for ec in range(EC):
                        enc_t_chunk = enc_t_pool.tile(
                            [128, NPAIR, S], ENC_DT, tag="enc_t"
                        )
                        nc.sync.dma_start(
                            out=enc_t_chunk,
                            in_=d_enc_t[b0 : b0 + NPAIR, ec].rearrange(
                                "b p s -> p b s"
                            ),
                        )
                        chunks.append(enc_t_chunk)
                    enc_t_tiles.append(chunks)
                    enc_n_tile = enc_n_pool.tile(
                        [128, NPAIR, SC, E], ENCN_DT, tag="enc_n"
                    )
                    nc.sync.dma_start(
                        out=enc_n_tile,
                        in_=d_enc_n[b0 : b0 + NPAIR].rearrange("b c p s -> p b c s"),
                    )
                    enc_n_tiles.append(enc_n_tile)

                scores_g = grp.tile([128, S], F32, tag="scores")
                nc.gpsimd.memset(scores_g, 0.0)
                madd_g = grp.tile([128, S], F32, tag="madd")
                nc.sync.dma_start(out=madd_g, in_=d_madd[g])

                # phases 1+2 per s-half; 4 batches packed into PE col groups
                for sh in range(SH):
                    sc_ps = ps.tile([128, 512], F32, tag="sc", bufs=2)
                    tanh_tiles = []
                    for j in range(G):
                        b = g * G + j
                        tanh_t = tanh_pool.tile([128, HC, 512], TANH_DT, tag="tanh")
                        enc_t_chunks = enc_t_tiles[j // NPAIR]
                        for hc in range(HC):
                            p1 = ps.tile([128, 512], F32, tag="p1")
                            for ec in range(EC):
                                nc.tensor.matmul(
                                    p1,
                                    w_enc_sb[:, ec, ts(hc, 128)],
                                    enc_t_chunks[ec][:, j % NPAIR, ts(sh, 512)],
                                    start=(ec == 0),
                                    stop=(ec == EC - 1),
                                )
                            nc.scalar.activation(
                                out=tanh_t[:, hc, :],
                                in_=p1,
                                func=AF.Tanh,
                                bias=ct_sb[:, hc, b : b + 1],
                            )
                        tanh_tiles.append(tanh_t)
                    for hc in range(HC):
                        for j in range(G):
                            nc.tensor.matmul(
                                sc_ps[32 * j : 32 * j + 1, :],
                                w_att_sb[:, hc : hc + 1],
                                tanh_tiles[j][:, hc, :],
                                start=(hc == 0),
                                stop=(hc == HC - 1),
                                tile_position=(0, 32 * j),
                                skip_group_check=True,
                            )
                    for j in range(G):
                        nc.vector.tensor_copy(
                            scores_g[32 * j : 32 * j + 1, ts(sh, 512)],
                            sc_ps[32 * j : 32 * j + 1, :],
                        )
                nc.vector.tensor_add(scores_g, scores_g, madd_g)

                # --- softmax (rows live on partitions {0,32,64,96}) ---
                negmax_g = grp.tile([128, 1], F32, tag="negmax")
                nc.vector.tensor_reduce(
                    negmax_g,
                    scores_g,
                    axis=mybir.AxisListType.X,
                    op=mybir.AluOpType.max,
                    negate=True,
                )
                esum_g = grp.tile([128, 1], F32, tag="esum")
                nc.scalar.activation(
                    out=scores_g,
                    in_=scores_g,
                    func=AF.Exp,
                    bias=negmax_g,
                    accum_out=esum_g,
                )
                rsum_g = grp.tile([128, 1], F32, tag="rsum")
                nc.vector.reciprocal(rsum_g, esum_g)
                wnorm_g = grp.tile([128, S], F32, tag="wnorm")
                nc.vector.tensor_scalar_mul(wnorm_g, in0=scores_g, scalar1=rsum_g)

                # weights output rows {0,32,64,96} -> DRAM
                nc.sync.dma_start(
                    out=d_w_out[g * G : (g + 1) * G, :],
                    in_=wnorm_g.rearrange("(a b) s -> a b s", b=32)[:, 0, :],
                )

                # --- transpose weights to [s, b] columns for phase 3 ---
                wt_sb = grp.tile([128, SC, 128], ENCN_DT, tag="wt_sb")
                for sc in range(SC):
                    wtp = ps.tile([128, 128], F32, tag="wt", bufs=1)
                    nc.tensor.transpose(wtp, wnorm_g[:, ts(sc, 128)], ident128)
                    nc.vector.tensor_copy(wt_sb[:, sc, :], wtp)

                # --- phase 3: context = sum_s w[s] * enc[s, :], col packed ---
                ctx_ps = ps.tile([128, 512], F32, tag="ctx")
                for sc in range(SC):
                    for j in range(G):
                        nc.tensor.matmul(
                            ctx_ps[32 * j : 32 * j + 1, :],
                            wt_sb[:, sc, 32 * j : 32 * j + 1],
                            enc_n_tiles[j // NPAIR][:, j % NPAIR, sc, :],
                            start=(sc == 0),
                            stop=(sc == SC - 1),
                            tile_position=(0, 32 * j),
                            skip_group_check=True,
                        )
                ctx_sp = grp.tile([128, D], F32, tag="ctx_sp")
                nc.gpsimd.memset(ctx_sp, 0.0)
                for j in range(G):
                    nc.vector.tensor_copy(
                        ctx_sp[32 * j : 32 * j + 1, :],
                        ctx_ps[32 * j : 32 * j + 1, :],
                    )
                # context rows -> transposed columns of ctxt_sb
                for ec in range(EC):
                    ctp = ps.tile([128, 128], F32, tag="wt", bufs=1)
                    nc.tensor.transpose(ctp, ctx_sp[:, ts(ec, 128)], ident128)
                    nc.vector.tensor_copy(
                        ctxt_sb[:, ec, g * G : (g + 1) * G],
                        ctp.rearrange("p (a b) -> p a b", b=32)[:, :, 0],
                    )

            # --- final: out = ctx @ W_ctx + b_ctx ---
            fin_ps = ps.tile([BC, D], F32, tag="fin", bufs=1)
            for ec in range(EC):
                nc.tensor.matmul(
                    fin_ps,
                    ctxt_sb[:, ec, :],
                    w_ctx_sb[:, ec, :],
                    start=(ec == 0),
                    stop=(ec == EC - 1),
                )
            nc.vector.tensor_add(out_sb, fin_ps, bias_ctx_sb)

            nc.sync.dma_start(out=d_ctx_out, in_=out_sb)

    if compile:
        nc.compile()
    return nc


def _get_nc():
    if "nc" not in _CACHE:
        _CACHE["nc"] = _build_nc()
    return _CACHE["nc"]


def _prepare_in_maps(
    encoder_outputs,
    decoder_state,
    attention_mask,
    W_enc,
    b_enc,
    W_dec,
    b_dec,
    W_att,
    b_att,
    W_ctx,
    b_ctx,
):
    import ml_dtypes

    bf16 = ml_dtypes.bfloat16

    enc = np.ascontiguousarray(np.asarray(encoder_outputs, dtype=np.float32))
    dec = np.ascontiguousarray(np.asarray(decoder_state, dtype=np.float32))
    mask = np.asarray(attention_mask)
    W_enc = np.asarray(W_enc, dtype=np.float32)
    b_enc = np.asarray(b_enc, dtype=np.float32)
    W_dec = np.asarray(W_dec, dtype=np.float32)
    b_dec = np.asarray(b_dec, dtype=np.float32)
    W_att = np.asarray(W_att, dtype=np.float32)
    W_ctx = np.asarray(W_ctx, dtype=np.float32)
    b_ctx = np.asarray(b_ctx, dtype=np.float32)

    # [B,S,E] -> transposed layout [B, EC, 128, S] and natural [B, SC, 128, E]
    enc_t = np.ascontiguousarray(enc.transpose(0, 2, 1)).astype(bf16)
    enc_t = enc_t.reshape(NCORES, BC, EC, 128, S)
    enc_n = enc.astype(bf16).reshape(NCORES, BC, SC, 128, E)

    w_enc_h = np.ascontiguousarray(
        W_enc.reshape(EC, 128, H).transpose(1, 0, 2)
    ).astype(bf16)
    w_att_h = np.ascontiguousarray(W_att[:, 0].reshape(HC, 128).T).astype(bf16)
    w_dec_h = _round_f32r(
        np.ascontiguousarray(W_dec.reshape(DC, 128, H).transpose(1, 0, 2))
    )
    w_ctx_h = _round_f32r(
        np.ascontiguousarray(W_ctx.reshape(EC, 128, D).transpose(1, 0, 2))
    )
    bias_de_h = np.ascontiguousarray(
        np.broadcast_to((b_enc + b_dec)[None, :], (BC, H))
    ).astype(np.float32)
    bias_ctx_h = np.ascontiguousarray(np.broadcast_to(b_ctx[None, :], (BC, D))).astype(
        np.float32
    )
    madd_rows = np.where(mask, np.float32(0.0), np.float32(NEG_INF)).astype(np.float32)
    madd_rows = madd_rows.reshape(NCORES, NG, G, S)
    madd = np.zeros((NCORES, NG, 128, S), np.float32)
    madd[:, :, ::32, :] = madd_rows

    # dec_t[core]: [128, DC, BC]
    dec_sh = dec.reshape(NCORES, BC, D)
    dec_t = np.ascontiguousarray(
        dec_sh.transpose(0, 2, 1).reshape(NCORES, DC, 128, BC).transpose(0, 2, 1, 3)
    )
    dec_t = _round_f32r(dec_t)

    in_maps = []
    for c in range(NCORES):
        in_maps.append(
            {
                "enc_t": enc_t[c],
                "enc_n": enc_n[c],
                "w_enc": w_enc_h,
                "w_att": w_att_h,
                "dec_t": dec_t[c],
                "w_dec": w_dec_h,
                "w_ctx": w_ctx_h,
                "bias_de": bias_de_h,
                "bias_ctx": bias_ctx_h,
                "madd": madd[c],
            }
        )
    return in_maps


def kernel(**inputs):
    from concourse.bass_utils import run_bass_kernel_spmd

    in_maps = _prepare_in_maps(**inputs)
    nc = _get_nc()
    trace = bool(int(os.environ.get("KERNEL_TRACE", "0")))
    if not trace:
        # the axon NTFF trace path needs a hook module this container lacks
        os.environ.setdefault("BASS_NEVER_TRACE", "1")
    kwargs = {}
    if trace:
        kwargs = {"trace": True, "tmpdir": os.environ.get("KERNEL_TRACE_DIR")}
    res = run_bass_kernel_spmd(nc, in_maps, core_ids=list(range(NCORES)), **kwargs)
    _CACHE["last_result"] = res

    context = np.concatenate([res.results[c]["ctx_out"] for c in range(NCORES)], axis=0)
    weights = np.concatenate([res.results[c]["w_out"] for c in range(NCORES)], axis=0)
    return context.astype(np.float32), weights.astype(np.float32)
